# revision 31
# baseline (speedup 1.0000x reference)
"""CfC Liquid Cell kernel for Trainium2 (Bass/Tile), 8 NeuronCores.

Sharding: data-parallel over batch (B=8 -> 1 batch element per core).

Single fused pipeline over time chunks of TC=256 (no DRAM scratch):
  per chunk c: XBAR DMA-transpose of x (bf16, host-cast) -> in_proj (bf16)
  -> depthwise conv (diag-matmul taps, bias folded into silu) -> head
  matmuls (block-diag 2-head 64x64 weights, shared across heads) ->
  elementwise gate algebra on DVE in bf16 (2x/4x modes) ->
  tensor_tensor_scan -> state_out -> z-gating -> out_proj with gated
  activations as the stationary operand (output time-major, DMA'd out).

Software pipelining: iteration i issues dma(i+1), in_proj/conv/heads/
scan(i), state_out+gating(i-1), out_proj(i-2) so cross-engine deps have
>= 1 chunk of slack.

sigmoid(u) = 0.5 + 0.5*tanh(u/2): the 0.5 on u is folded into the
tau/decay weights+biases on the host; DVE affine recovers sigmoid.
"""

import numpy as np

B, S, H = 8, 2048, 1024
NH, HD, NS, K = 16, 64, 64, 4
N_CORES = 8
P = 128
TC = 256            # time chunk
NC = S // TC        # 8

_CACHE = {}


def _build_program():
    import concourse.bacc as bacc
    import concourse.mybir as mybir
    import concourse.tile as tile

    F32 = mybir.dt.float32
    BF16 = mybir.dt.bfloat16
    AF = mybir.ActivationFunctionType
    ALU = mybir.AluOpType

    nc = bacc.Bacc("TRN2", target_bir_lowering=False, debug=False)

    x_d = nc.dram_tensor("x", (S, H), BF16, kind="ExternalInput").ap()
    w_in_d = nc.dram_tensor("w_in", (P, 8, 2 * H), BF16, kind="ExternalInput").ap()
    w_out_d = nc.dram_tensor("w_out", (P, 8, H), BF16, kind="ExternalInput").ap()
    cdiag_d = nc.dram_tensor("cdiag", (P, 8, 4, P), BF16, kind="ExternalInput").ap()
    blk_d = nc.dram_tensor("blk", (P, 6, P), BF16, kind="ExternalInput").ap()
    bias_d = nc.dram_tensor("bias", (P, 6), F32, kind="ExternalInput").ap()
    cbias_d = nc.dram_tensor("cbias", (P, 8), F32, kind="ExternalInput").ap()
    y_d = nc.dram_tensor("y", (S, H), F32, kind="ExternalOutput").ap()

    with tile.TileContext(nc) as tc:
        with tc.tile_pool(name="const", bufs=1) as cpool:
            w_in = cpool.tile([P, 8, 2 * H], BF16)
            nc.sync.dma_start(w_in[:], w_in_d[:])
            cdiag = cpool.tile([P, 8, 4, P], BF16)
            nc.sync.dma_start(cdiag[:], cdiag_d[:])
            blk = cpool.tile([P, 6, P], BF16)
            nc.sync.dma_start(blk[:], blk_d[:])
            bias = cpool.tile([P, 6], F32)
            nc.sync.dma_start(bias[:], bias_d[:])
            cbias = cpool.tile([P, 8], F32)
            nc.sync.dma_start(cbias[:], cbias_d[:])
            w_out = cpool.tile([P, 8, H], BF16)  # dma issued after xT(0..1)

            with tc.tile_pool(name="pxT", bufs=3) as pxT, \
                 tc.tile_pool(name="pxp", bufs=2) as pxp, \
                 tc.tile_pool(name="pzs", bufs=3) as pzs, \
                 tc.tile_pool(name="pxh", bufs=2) as pxh, \
                 tc.tile_pool(name="pbb", bufs=2) as pbb, \
                 tc.tile_pool(name="work", bufs=9) as work, \
                 tc.tile_pool(name="ph", bufs=2) as phh, \
                 tc.tile_pool(name="pog", bufs=4) as pog, \
                 tc.tile_pool(name="pyb", bufs=2) as pyb, \
                 tc.tile_pool(name="psA", bufs=2, space="PSUM") as psA, \
                 tc.tile_pool(name="psG", bufs=2, space="PSUM") as psG, \
                 tc.tile_pool(name="psY", bufs=2, space="PSUM") as psY:

                def dma_in(c):
                    """XBAR DMA transpose: x chunk -> feature-major bf16 xT.
                    Runs on the Activation HWDGE queue so the weight DMAs on
                    the SP queue don't delay the first chunks."""
                    xT = pxT.tile([P, 8, TC], BF16, tag="xT", name="xT")
                    nc.scalar.dma_start_transpose(
                        xT[:], x_d[c * TC:(c + 1) * TC, :])
                    return xT

                def in_proj(c, xT, xp_prev):
                    """xz = x @ W_in; x_path (jt 0..7) first, then z."""
                    xp = pxp.tile([P, 8, 3 + TC], BF16, tag="xp", name="xp")
                    if c == 0:
                        nc.vector.memset(xp[:, :, :3], 0.0)
                    else:
                        nc.vector.tensor_copy(xp[:, :, :3],
                                              xp_prev[:, :, TC:TC + 3])
                    zs = pzs.tile([P, 8, TC], BF16, tag="zs", name="zs")
                    # z half first: its ACT readers are off the critical
                    # path, so the psA slots recycled by the next chunk are
                    # freed by the fast DVE xp copies instead.
                    for jp in (4, 5, 6, 7, 0, 1, 2, 3):
                        jt = 2 * jp
                        pm = psA.tile([P, 2, TC], F32, tag="psA", name="pm")
                        n = 0
                        for kt in range(8):
                            for j in range(2):
                                nc.tensor.matmul(
                                    pm[:, j, :],
                                    w_in[:, kt, (jt + j) * P:(jt + j + 1) * P],
                                    xT[:, kt, :],
                                    start=(n == 0), stop=(n == 15),
                                    skip_group_check=True)
                                n += 1
                        if jp < 4:
                            nc.vector.tensor_copy(xp[:, jt:jt + 2, 3:], pm[:])
                        else:
                            nc.scalar.activation(zs[:, jt - 8:jt - 6, :],
                                                 pm[:], AF.Silu)
                    return xp, zs

                def conv(c, xp):
                    """depthwise causal conv + silu (bias in the act)."""
                    xh = pxh.tile([P, 8, TC], BF16, tag="xh", name="xh")
                    for cp in range(4):          # pairs of ct
                        ct = 2 * cp
                        pc = psA.tile([P, 2, TC], F32, tag="psA", name="pc")
                        n = 0
                        for tap in range(4):
                            for j in range(2):
                                nc.tensor.matmul(
                                    pc[:, j, :], cdiag[:, ct + j, tap, :],
                                    xp[:, ct + j, tap:tap + TC],
                                    start=(n == 0), stop=(n == 7),
                                    skip_group_check=True)
                                n += 1
                        for j in range(2):
                            nc.scalar.activation(
                                xh[:, ct + j, :], pc[:, j, :], AF.Silu,
                                bias=cbias[:, ct + j:ct + j + 1])
                    return xh

                def stage(widx, rhs_t, out_t, func, bias_col):
                    for m in range(2):
                        pg = psG.tile([P, 4, TC], F32, tag="psG", name="pg")
                        for q in range(2):
                            nc.tensor.matmul(
                                pg[:, 2 * q:2 * q + 2, :], blk[:, widx, :],
                                rhs_t[:, 4 * m + 2 * q:4 * m + 2 * q + 2, :],
                                start=True, stop=True)
                        nc.scalar.activation(
                            out_t[:, 4 * m:4 * m + 4, :], pg[:], func,
                            bias=bias[:, bias_col:bias_col + 1])

                def heads_scan(c, xh, h_prev):
                    bb = pbb.tile([P, 8, TC], BF16, tag="bb", name="bb")
                    stage(0, xh, bb, AF.Silu, 0)
                    f1 = work.tile([P, 8, TC], BF16, tag="work", name="f1")
                    stage(1, bb, f1, AF.Tanh, 1)
                    f2 = work.tile([P, 8, TC], BF16, tag="work", name="f2")
                    stage(2, bb, f2, AF.Tanh, 2)
                    ta = work.tile([P, 8, TC], BF16, tag="work", name="ta")
                    stage(3, bb, ta, AF.Tanh, 3)
                    tg = work.tile([P, 8, TC], BF16, tag="work", name="tg")
                    stage(4, bb, tg, AF.Tanh, 4)

                    # candidate*2 = (f1+f2) + a*(f2-f1); u = c2 * (1-g)/4
                    sm = work.tile([P, 8, TC], BF16, tag="work", name="sm")
                    nc.vector.tensor_tensor(sm[:], f1[:], f2[:], ALU.add)
                    dl = work.tile([P, 8, TC], BF16, tag="work", name="dl")
                    nc.vector.tensor_tensor(dl[:], f2[:], f1[:], ALU.subtract)
                    tt = work.tile([P, 8, TC], BF16, tag="work", name="tt")
                    nc.vector.tensor_tensor(tt[:], ta[:], dl[:], ALU.mult)
                    c2 = work.tile([P, 8, TC], BF16, tag="work", name="c2")
                    nc.vector.tensor_tensor(c2[:], sm[:], tt[:], ALU.add)
                    wq = work.tile([P, 8, TC], BF16, tag="work", name="wq")
                    nc.vector.tensor_scalar(wq[:], tg[:], -0.25, 0.25,
                                            ALU.mult, ALU.add)
                    uu = work.tile([P, 8, TC], BF16, tag="work", name="uu")
                    nc.vector.tensor_tensor(uu[:], c2[:], wq[:], ALU.mult)
                    dd = work.tile([P, 8, TC], BF16, tag="work", name="dd")
                    nc.vector.tensor_scalar(dd[:], tg[:], 0.5, 0.5,
                                            ALU.mult, ALU.add)

                    h = phh.tile([P, 8, TC], BF16, tag="h", name="h")
                    for hp in range(8):
                        init = 0.0 if c == 0 else h_prev[:, hp, TC - 1:TC]
                        nc.vector.tensor_tensor_scan(
                            h[:, hp, :], dd[:, hp, :], uu[:, hp, :], init,
                            ALU.mult, ALU.add)
                    return h

                def stategate(c, h, zs):
                    oseq = pog.tile([P, 8, TC], BF16, tag="og", name="oseq")
                    stage(5, h, oseq, AF.Identity, 5)
                    gh = pog.tile([P, 8, TC], BF16, tag="og", name="gh")
                    nc.vector.tensor_tensor(gh[:], oseq[:], zs[:], ALU.mult)
                    return gh

                def tail_chunk(c, h, zs):
                    """Final chunk: interleave state_out/gating/out_proj by
                    128-step time block to shorten the drain chain."""
                    oseq = pog.tile([P, 8, TC], BF16, tag="og", name="oseq")
                    gh = pog.tile([P, 8, TC], BF16, tag="og", name="gh")
                    for tb in range(TC // P):
                        ts = slice(tb * P, (tb + 1) * P)
                        for m in range(2):
                            pg = psG.tile([P, 4, P], F32, tag="psG", name="pg")
                            for q in range(2):
                                nc.tensor.matmul(
                                    pg[:, 2 * q:2 * q + 2, :], blk[:, 5, :],
                                    h[:, 4 * m + 2 * q:4 * m + 2 * q + 2, ts],
                                    start=True, stop=True)
                            nc.scalar.activation(
                                oseq[:, 4 * m:4 * m + 4, ts], pg[:],
                                AF.Identity, bias=bias[:, 5:6])
                        nc.vector.tensor_tensor(gh[:, :, ts], oseq[:, :, ts],
                                                zs[:, :, ts], ALU.mult)
                        ysb = pyb.tile([P, H], F32, tag="ysb", name="ysb")
                        for hf in range(2):
                            py = psY.tile([P, H // 2], F32, tag="psY", name="py")
                            for kt in range(8):
                                nc.tensor.matmul(
                                    py[:], gh[:, kt, ts],
                                    w_out[:, kt, hf * 512:(hf + 1) * 512],
                                    start=(kt == 0), stop=(kt == 7))
                            if hf == 0:
                                nc.scalar.activation(
                                    ysb[:, 0:512], py[:], AF.Copy)
                            else:
                                nc.vector.tensor_copy(ysb[:, 512:1024], py[:])
                        nc.sync.dma_start(
                            y_d[(c * 2 + tb) * P:(c * 2 + tb + 1) * P, :],
                            ysb[:])

                def out_proj(c, gh):
                    for tb in range(TC // P):
                        ysb = pyb.tile([P, H], F32, tag="ysb", name="ysb")
                        for hf in range(2):
                            py = psY.tile([P, H // 2], F32, tag="psY", name="py")
                            for kt in range(8):
                                nc.tensor.matmul(
                                    py[:], gh[:, kt, tb * P:(tb + 1) * P],
                                    w_out[:, kt, hf * 512:(hf + 1) * 512],
                                    start=(kt == 0), stop=(kt == 7))
                            if hf == 0:
                                nc.scalar.activation(
                                    ysb[:, 0:512], py[:], AF.Copy)
                            else:
                                nc.vector.tensor_copy(ysb[:, 512:1024], py[:])
                        nc.sync.dma_start(
                            y_d[(c * 2 + tb) * P:(c * 2 + tb + 1) * P, :],
                            ysb[:])

                # software pipeline; x DMA-transpose prefetched 1 ahead
                xT_cur = dma_in(0)
                xT_next = dma_in(1)
                nc.sync.dma_start(w_out[:], w_out_d[:])
                xp_prev = None
                h_prev = None
                sg_pend = None   # (c, h, zs)
                op_pend = None   # (c, gh)
                for i in range(NC + 1):
                    if i < NC:
                        xp, zs = in_proj(i, xT_cur, xp_prev)
                        xp_prev = xp
                        xh = conv(i, xp)
                        h = heads_scan(i, xh, h_prev)
                        h_prev = h
                        sg_next = (i, h, zs)
                    else:
                        sg_next = None
                    if i + 2 < NC:
                        xT_next2 = dma_in(i + 2)
                    else:
                        xT_next2 = None
                    if op_pend is not None:
                        out_proj(*op_pend)
                    if sg_pend is not None:
                        if sg_pend[0] == NC - 1:
                            tail_chunk(*sg_pend)
                            op_next = None
                        else:
                            op_next = (sg_pend[0],
                                       stategate(sg_pend[0], sg_pend[1],
                                                 sg_pend[2]))
                    else:
                        op_next = None
                    sg_pend = sg_next
                    op_pend = op_next
                    xT_cur, xT_next = xT_next, xT_next2

    nc.compile()
    return nc


def _prep_shared(inputs):
    """Host-side preprocessing of the shared (weight) tensors."""
    import ml_dtypes
    f32 = np.float32
    bf = ml_dtypes.bfloat16
    in_proj_w = np.asarray(inputs["in_proj_w"], f32)
    conv_w = np.asarray(inputs["conv_w"], f32)
    conv_b = np.asarray(inputs["conv_b"], f32)

    w_in = in_proj_w.reshape(8, P, 2 * H).transpose(1, 0, 2)
    w_out = np.asarray(inputs["out_proj_w"], f32).reshape(8, P, H).transpose(1, 0, 2)

    cdiag = np.zeros((8, 4, P, P), f32)
    rng = np.arange(P)
    for ct in range(8):
        for tap in range(K):
            cdiag[ct, tap, rng, rng] = conv_w[ct * P:(ct + 1) * P, 0, tap]
    cdiag = cdiag.transpose(2, 0, 1, 3)  # (P, 8, 4, P)
    cbias = conv_b.reshape(8, P).T  # (P, 8)

    def blk2(w):
        o = np.zeros((P, P), f32)
        o[:64, :64] = w
        o[64:, 64:] = w
        return o

    blk = np.stack([
        blk2(np.asarray(inputs["bb_w"], f32)),
        blk2(np.asarray(inputs["f1_w"], f32)),
        blk2(np.asarray(inputs["f2_w"], f32)),
        blk2(np.asarray(inputs["tau_a_w"], f32) * 0.5),
        blk2(np.asarray(inputs["decay_w"], f32) * 0.5),
        blk2(np.asarray(inputs["state_out_w"], f32)),
    ], axis=1)  # (P, 6, P)

    def t2(v):
        return np.tile(np.asarray(v, f32), 2)

    bias = np.stack([
        t2(inputs["bb_b"]),
        t2(inputs["f1_b"]),
        t2(inputs["f2_b"]),
        0.5 * (t2(inputs["tau_a_b"]) + t2(inputs["tau_b"])),
        0.5 * t2(inputs["decay_b"]),
        t2(inputs["state_out_b"]),
    ], axis=1)  # (P, 6)

    return {
        "w_in": np.ascontiguousarray(w_in).astype(bf),
        "w_out": np.ascontiguousarray(w_out).astype(bf),
        "cdiag": np.ascontiguousarray(cdiag).astype(bf),
        "blk": np.ascontiguousarray(blk).astype(bf),
        "bias": np.ascontiguousarray(bias),
        "cbias": np.ascontiguousarray(cbias),
    }


def _in_maps(inputs):
    import ml_dtypes
    shared = _prep_shared(inputs)
    x = np.asarray(inputs["x"], np.float32).astype(ml_dtypes.bfloat16)
    in_maps = []
    for b in range(N_CORES):
        m = dict(shared)
        m["x"] = np.ascontiguousarray(x[b])
        in_maps.append(m)
    return in_maps


def kernel(**inputs) -> np.ndarray:
    from concourse import bass_utils

    if "nc" not in _CACHE:
        _CACHE["nc"] = _build_program()
    nc = _CACHE["nc"]

    res = bass_utils.run_bass_kernel_spmd(nc, _in_maps(inputs),
                                          core_ids=list(range(N_CORES)))
    out = np.stack([res.results[b]["y"] for b in range(N_CORES)], axis=0)
    return out.astype(np.float32)


# revision 32
# speedup vs baseline: 1.0507x; 1.0507x over previous
"""CfC Liquid Cell kernel for Trainium2 (Bass/Tile), 8 NeuronCores.

Sharding: data-parallel over batch (B=8 -> 1 batch element per core).

Single fused pipeline over time chunks of TC=256 (no DRAM scratch):
  per chunk c: XBAR DMA-transpose of x (bf16, host-cast) -> in_proj (bf16)
  -> depthwise conv (diag-matmul taps, bias folded into silu) -> head
  matmuls (block-diag 2-head 64x64 weights, shared across heads) ->
  elementwise gate algebra on DVE in bf16 (2x/4x modes) ->
  tensor_tensor_scan -> state_out -> z-gating -> out_proj with gated
  activations as the stationary operand (output time-major, DMA'd out).

Software pipelining: iteration i issues dma(i+1), in_proj/conv/heads/
scan(i), state_out+gating(i-1), out_proj(i-2) so cross-engine deps have
>= 1 chunk of slack.

sigmoid(u) = 0.5 + 0.5*tanh(u/2): the 0.5 on u is folded into the
tau/decay weights+biases on the host; DVE affine recovers sigmoid.
"""

import numpy as np

B, S, H = 8, 2048, 1024
NH, HD, NS, K = 16, 64, 64, 4
N_CORES = 8
P = 128
TC = 256            # time chunk
NC = S // TC        # 8

_CACHE = {}


def _build_program():
    import concourse.bacc as bacc
    import concourse.mybir as mybir
    import concourse.tile as tile

    F32 = mybir.dt.float32
    BF16 = mybir.dt.bfloat16
    AF = mybir.ActivationFunctionType
    ALU = mybir.AluOpType

    nc = bacc.Bacc("TRN2", target_bir_lowering=False, debug=False)

    x_d = nc.dram_tensor("x", (S, H), BF16, kind="ExternalInput").ap()
    w_in_d = nc.dram_tensor("w_in", (P, 8, 2 * H), BF16, kind="ExternalInput").ap()
    w_out_d = nc.dram_tensor("w_out", (P, 8, H), BF16, kind="ExternalInput").ap()
    cdiag_d = nc.dram_tensor("cdiag", (P, 8, 4, P), BF16, kind="ExternalInput").ap()
    blk_d = nc.dram_tensor("blk", (P, 6, P), BF16, kind="ExternalInput").ap()
    bias_d = nc.dram_tensor("bias", (P, 6), F32, kind="ExternalInput").ap()
    cbias_d = nc.dram_tensor("cbias", (P, 8), F32, kind="ExternalInput").ap()
    y_d = nc.dram_tensor("y", (S, H), F32, kind="ExternalOutput").ap()

    with tile.TileContext(nc) as tc:
        with tc.tile_pool(name="const", bufs=1) as cpool:
            w_in = cpool.tile([P, 8, 2 * H], BF16)
            nc.sync.dma_start(w_in[:], w_in_d[:])
            cdiag = cpool.tile([P, 8, 4, P], BF16)
            nc.sync.dma_start(cdiag[:], cdiag_d[:])
            blk = cpool.tile([P, 6, P], BF16)
            nc.sync.dma_start(blk[:], blk_d[:])
            bias = cpool.tile([P, 6], F32)
            nc.sync.dma_start(bias[:], bias_d[:])
            cbias = cpool.tile([P, 8], F32)
            nc.sync.dma_start(cbias[:], cbias_d[:])
            w_out = cpool.tile([P, 8, H], BF16)  # dma issued after xT(0..1)

            with tc.tile_pool(name="pxT", bufs=3) as pxT, \
                 tc.tile_pool(name="pxp", bufs=2) as pxp, \
                 tc.tile_pool(name="pzs", bufs=3) as pzs, \
                 tc.tile_pool(name="pxh", bufs=2) as pxh, \
                 tc.tile_pool(name="pbb", bufs=2) as pbb, \
                 tc.tile_pool(name="work", bufs=9) as work, \
                 tc.tile_pool(name="ph", bufs=2) as phh, \
                 tc.tile_pool(name="pog", bufs=4) as pog, \
                 tc.tile_pool(name="pyb", bufs=2) as pyb, \
                 tc.tile_pool(name="psA", bufs=2, space="PSUM") as psA, \
                 tc.tile_pool(name="psG", bufs=2, space="PSUM") as psG, \
                 tc.tile_pool(name="psY", bufs=2, space="PSUM") as psY:

                def dma_in(c):
                    """XBAR DMA transpose: x chunk -> feature-major bf16 xT.
                    Runs on the Activation HWDGE queue so the weight DMAs on
                    the SP queue don't delay the first chunks."""
                    xT = pxT.tile([P, 8, TC], BF16, tag="xT", name="xT")
                    nc.scalar.dma_start_transpose(
                        xT[:], x_d[c * TC:(c + 1) * TC, :])
                    return xT

                def in_proj(c, xT, xp_prev):
                    """xz = x @ W_in; x_path (jt 0..7) first, then z."""
                    xp = pxp.tile([P, 8, 3 + TC], BF16, tag="xp", name="xp")
                    if c == 0:
                        nc.vector.memset(xp[:, :, :3], 0.0)
                    else:
                        nc.vector.tensor_copy(xp[:, :, :3],
                                              xp_prev[:, :, TC:TC + 3])
                    zs = pzs.tile([P, 8, TC], BF16, tag="zs", name="zs")
                    for jp in range(8):          # pairs of jt
                        jt = 2 * jp
                        pm = psA.tile([P, 2, TC], F32, tag="psA", name="pm")
                        n = 0
                        for kt in range(8):
                            for j in range(2):
                                nc.tensor.matmul(
                                    pm[:, j, :],
                                    w_in[:, kt, (jt + j) * P:(jt + j + 1) * P],
                                    xT[:, kt, :],
                                    start=(n == 0), stop=(n == 15),
                                    skip_group_check=True)
                                n += 1
                        if jp < 4:
                            nc.vector.tensor_copy(xp[:, jt:jt + 2, 3:], pm[:])
                        else:
                            nc.scalar.activation(zs[:, jt - 8:jt - 6, :],
                                                 pm[:], AF.Silu)
                    return xp, zs

                def conv(c, xp):
                    """depthwise causal conv + silu (bias in the act)."""
                    xh = pxh.tile([P, 8, TC], BF16, tag="xh", name="xh")
                    for cp in range(4):          # pairs of ct
                        ct = 2 * cp
                        pc = psA.tile([P, 2, TC], F32, tag="psA", name="pc")
                        n = 0
                        for tap in range(4):
                            for j in range(2):
                                nc.tensor.matmul(
                                    pc[:, j, :], cdiag[:, ct + j, tap, :],
                                    xp[:, ct + j, tap:tap + TC],
                                    start=(n == 0), stop=(n == 7),
                                    skip_group_check=True)
                                n += 1
                        for j in range(2):
                            nc.scalar.activation(
                                xh[:, ct + j, :], pc[:, j, :], AF.Silu,
                                bias=cbias[:, ct + j:ct + j + 1])
                    return xh

                def stage(widx, rhs_t, out_t, func, bias_col):
                    for m in range(2):
                        pg = psG.tile([P, 4, TC], F32, tag="psG", name="pg")
                        for q in range(2):
                            nc.tensor.matmul(
                                pg[:, 2 * q:2 * q + 2, :], blk[:, widx, :],
                                rhs_t[:, 4 * m + 2 * q:4 * m + 2 * q + 2, :],
                                start=True, stop=True)
                        nc.scalar.activation(
                            out_t[:, 4 * m:4 * m + 4, :], pg[:], func,
                            bias=bias[:, bias_col:bias_col + 1])

                def heads_scan(c, xh, h_prev):
                    bb = pbb.tile([P, 8, TC], BF16, tag="bb", name="bb")
                    stage(0, xh, bb, AF.Silu, 0)
                    f1 = work.tile([P, 8, TC], BF16, tag="work", name="f1")
                    stage(1, bb, f1, AF.Tanh, 1)
                    f2 = work.tile([P, 8, TC], BF16, tag="work", name="f2")
                    stage(2, bb, f2, AF.Tanh, 2)
                    ta = work.tile([P, 8, TC], BF16, tag="work", name="ta")
                    stage(3, bb, ta, AF.Tanh, 3)
                    tg = work.tile([P, 8, TC], BF16, tag="work", name="tg")
                    stage(4, bb, tg, AF.Tanh, 4)

                    # candidate*2 = (f1+f2) + a*(f2-f1); u = c2 * (1-g)/4
                    sm = work.tile([P, 8, TC], BF16, tag="work", name="sm")
                    nc.vector.tensor_tensor(sm[:], f1[:], f2[:], ALU.add)
                    dl = work.tile([P, 8, TC], BF16, tag="work", name="dl")
                    nc.vector.tensor_tensor(dl[:], f2[:], f1[:], ALU.subtract)
                    tt = work.tile([P, 8, TC], BF16, tag="work", name="tt")
                    nc.vector.tensor_tensor(tt[:], ta[:], dl[:], ALU.mult)
                    c2 = work.tile([P, 8, TC], BF16, tag="work", name="c2")
                    nc.vector.tensor_tensor(c2[:], sm[:], tt[:], ALU.add)
                    wq = work.tile([P, 8, TC], BF16, tag="work", name="wq")
                    nc.vector.tensor_scalar(wq[:], tg[:], -0.25, 0.25,
                                            ALU.mult, ALU.add)
                    uu = work.tile([P, 8, TC], BF16, tag="work", name="uu")
                    nc.vector.tensor_tensor(uu[:], c2[:], wq[:], ALU.mult)
                    dd = work.tile([P, 8, TC], BF16, tag="work", name="dd")
                    nc.vector.tensor_scalar(dd[:], tg[:], 0.5, 0.5,
                                            ALU.mult, ALU.add)

                    h = phh.tile([P, 8, TC], BF16, tag="h", name="h")
                    for hp in range(8):
                        init = 0.0 if c == 0 else h_prev[:, hp, TC - 1:TC]
                        nc.vector.tensor_tensor_scan(
                            h[:, hp, :], dd[:, hp, :], uu[:, hp, :], init,
                            ALU.mult, ALU.add)
                    return h

                def stategate(c, h, zs):
                    oseq = pog.tile([P, 8, TC], BF16, tag="og", name="oseq")
                    stage(5, h, oseq, AF.Identity, 5)
                    gh = pog.tile([P, 8, TC], BF16, tag="og", name="gh")
                    nc.vector.tensor_tensor(gh[:], oseq[:], zs[:], ALU.mult)
                    return gh

                def out_proj(c, gh):
                    for tb in range(TC // P):
                        ysb = pyb.tile([P, H], F32, tag="ysb", name="ysb")
                        for hf in range(2):
                            py = psY.tile([P, H // 2], F32, tag="psY", name="py")
                            for kt in range(8):
                                nc.tensor.matmul(
                                    py[:], gh[:, kt, tb * P:(tb + 1) * P],
                                    w_out[:, kt, hf * 512:(hf + 1) * 512],
                                    start=(kt == 0), stop=(kt == 7))
                            if hf == 0:
                                nc.scalar.activation(
                                    ysb[:, 0:512], py[:], AF.Copy)
                            else:
                                nc.vector.tensor_copy(ysb[:, 512:1024], py[:])
                        nc.sync.dma_start(
                            y_d[(c * 2 + tb) * P:(c * 2 + tb + 1) * P, :],
                            ysb[:])

                # software pipeline; x DMA-transpose prefetched 1 ahead
                xT_cur = dma_in(0)
                xT_next = dma_in(1)
                nc.sync.dma_start(w_out[:], w_out_d[:])
                xp_prev = None
                h_prev = None
                sg_pend = None   # (c, h, zs)
                op_pend = None   # (c, gh)
                for i in range(NC + 2):
                    if i < NC:
                        xp, zs = in_proj(i, xT_cur, xp_prev)
                        xp_prev = xp
                        xh = conv(i, xp)
                        h = heads_scan(i, xh, h_prev)
                        h_prev = h
                        sg_next = (i, h, zs)
                    else:
                        sg_next = None
                    if i + 2 < NC:
                        xT_next2 = dma_in(i + 2)
                    else:
                        xT_next2 = None
                    if sg_pend is not None:
                        op_next = (sg_pend[0], stategate(sg_pend[0],
                                                         sg_pend[1], sg_pend[2]))
                    else:
                        op_next = None
                    if op_pend is not None:
                        out_proj(*op_pend)
                    sg_pend = sg_next
                    op_pend = op_next
                    xT_cur, xT_next = xT_next, xT_next2

    nc.compile()
    return nc


def _prep_shared(inputs):
    """Host-side preprocessing of the shared (weight) tensors."""
    import ml_dtypes
    f32 = np.float32
    bf = ml_dtypes.bfloat16
    in_proj_w = np.asarray(inputs["in_proj_w"], f32)
    conv_w = np.asarray(inputs["conv_w"], f32)
    conv_b = np.asarray(inputs["conv_b"], f32)

    w_in = in_proj_w.reshape(8, P, 2 * H).transpose(1, 0, 2)
    w_out = np.asarray(inputs["out_proj_w"], f32).reshape(8, P, H).transpose(1, 0, 2)

    cdiag = np.zeros((8, 4, P, P), f32)
    rng = np.arange(P)
    for ct in range(8):
        for tap in range(K):
            cdiag[ct, tap, rng, rng] = conv_w[ct * P:(ct + 1) * P, 0, tap]
    cdiag = cdiag.transpose(2, 0, 1, 3)  # (P, 8, 4, P)
    cbias = conv_b.reshape(8, P).T  # (P, 8)

    def blk2(w):
        o = np.zeros((P, P), f32)
        o[:64, :64] = w
        o[64:, 64:] = w
        return o

    blk = np.stack([
        blk2(np.asarray(inputs["bb_w"], f32)),
        blk2(np.asarray(inputs["f1_w"], f32)),
        blk2(np.asarray(inputs["f2_w"], f32)),
        blk2(np.asarray(inputs["tau_a_w"], f32) * 0.5),
        blk2(np.asarray(inputs["decay_w"], f32) * 0.5),
        blk2(np.asarray(inputs["state_out_w"], f32)),
    ], axis=1)  # (P, 6, P)

    def t2(v):
        return np.tile(np.asarray(v, f32), 2)

    bias = np.stack([
        t2(inputs["bb_b"]),
        t2(inputs["f1_b"]),
        t2(inputs["f2_b"]),
        0.5 * (t2(inputs["tau_a_b"]) + t2(inputs["tau_b"])),
        0.5 * t2(inputs["decay_b"]),
        t2(inputs["state_out_b"]),
    ], axis=1)  # (P, 6)

    return {
        "w_in": np.ascontiguousarray(w_in).astype(bf),
        "w_out": np.ascontiguousarray(w_out).astype(bf),
        "cdiag": np.ascontiguousarray(cdiag).astype(bf),
        "blk": np.ascontiguousarray(blk).astype(bf),
        "bias": np.ascontiguousarray(bias),
        "cbias": np.ascontiguousarray(cbias),
    }


def _in_maps(inputs):
    import ml_dtypes
    shared = _prep_shared(inputs)
    x = np.asarray(inputs["x"], np.float32).astype(ml_dtypes.bfloat16)
    in_maps = []
    for b in range(N_CORES):
        m = dict(shared)
        m["x"] = np.ascontiguousarray(x[b])
        in_maps.append(m)
    return in_maps


def kernel(**inputs) -> np.ndarray:
    from concourse import bass_utils

    if "nc" not in _CACHE:
        _CACHE["nc"] = _build_program()
    nc = _CACHE["nc"]

    res = bass_utils.run_bass_kernel_spmd(nc, _in_maps(inputs),
                                          core_ids=list(range(N_CORES)))
    out = np.stack([res.results[b]["y"] for b in range(N_CORES)], axis=0)
    return out.astype(np.float32)
